# revision 3
# baseline (speedup 1.0000x reference)
"""Trainium2 Bass kernel for nn_DWTModelFullBand.

The reference computes a 2-level 2D Haar DWT (wavedec2) and immediately
inverts it (waverec2) reusing the cached level-1 detail bands. idwt2 is the
exact algebraic inverse of dwt2 (orthonormal Haar), so the whole pipeline is
the identity map on x; in fp32 the reference output differs from x only by
rounding noise (~6e-8 relative L2), the same magnitude any re-implementation
with different operation order would produce. The memory-roofline kernel is
therefore a pure copy: read x once from HBM, write it once.

Sharding: pure data parallel over batch — B=32 split as 4 samples per core
across 8 NeuronCores; each core DMA-copies its 12.58 MB shard DRAM->DRAM.

DMA schedule: the two HWDGE rings (Sync + Scalar queues) deal descriptors
round-robin across the 16 SDMA engines. Engine 15 (E79) also hosts the ring
processing and sustains only ~79% of the other engines' ~21 GB/s, so a
uniform deal makes it straggle ~10 us. Each queue therefore issues rounds of
exactly 16 descriptors shaped [16 rows x A][15 rows x B][1 row x Y] with
Y ~= 0.79*B: the single Y row lands on deal position 15 (E79) every round,
shrinking its share so all 16 engines finish together. Element counts are
chosen so bass's AP splitter reproduces these exact row shapes (A free,
B odd -> 15 rows, Y with smallest prime factor > 16 -> single descriptor).
Only the final dma_start of each queue carries the semaphore increment, so
no mid-stream 4-byte sem packets perturb the deal or waste engine slots.
"""

import numpy as np

_B, _C, _H, _W = 32, 3, 512, 512
_NCORES = 8
_BS = _B // _NCORES  # batch shard per core
_SHARD_ELEMS = _BS * _C * _H * _W  # 3,145,728 f32 = 12.58 MB

# Per-queue stream: _K rounds of (16*_A + 15*_B + _Y) f32 elements.
# 2 queues x _K x round == _SHARD_ELEMS.
_K = 4
_A, _B_ROW, _Y = 12445, 12449, 7361
_ROUND = 16 * _A + 15 * _B_ROW + _Y
assert 2 * _K * _ROUND == _SHARD_ELEMS

_cache = {}


def _build_nc():
    import concourse.bass as bass
    import concourse.mybir as mybir

    nc = bass.Bass()
    x = nc.declare_dram_parameter("x", [_SHARD_ELEMS], mybir.dt.float32, isOutput=False)
    y = nc.declare_dram_parameter("y", [_SHARD_ELEMS], mybir.dt.float32, isOutput=True)

    # Every HWDGE dma_start must carry sync info ("DGE must have sync info"),
    # and each one emits 16 four-byte sem packets (one per engine) — which
    # advance the deal pointer by 16 = 0 (mod 16), so the skew layout holds.
    half = _K * _ROUND
    n_dma = 0
    with nc.semaphore("dma_sem") as dma_sem:
        for qi, eng in enumerate((nc.sync, nc.scalar)):
            base = qi * half
            for k in range(_K):
                o = base + k * _ROUND
                for p in (16 * _A, 15 * _B_ROW, _Y):
                    sl = slice(o, o + p)
                    eng.dma_start(out=y[sl], in_=x[sl]).then_inc(dma_sem, 16)
                    o += p
                    n_dma += 1
        nc.sync.wait_ge(dma_sem, 16 * n_dma)

    return nc


def _get_nc():
    if "nc" not in _cache:
        _cache["nc"] = _build_nc()
    return _cache["nc"]


def kernel(x: np.ndarray, *, _trace: bool = False, _tmpdir: str | None = None) -> np.ndarray:
    from concourse.bass_utils import run_bass_kernel_spmd

    x = np.ascontiguousarray(np.asarray(x), dtype=np.float32)
    assert x.shape == (_B, _C, _H, _W), x.shape

    nc = _get_nc()
    shards = x.reshape(_NCORES, _SHARD_ELEMS)
    in_maps = [{"x": shards[i]} for i in range(_NCORES)]
    res = run_bass_kernel_spmd(
        nc, in_maps, core_ids=list(range(_NCORES)), trace=_trace, tmpdir=_tmpdir
    )
    _cache["last_result"] = res
    out = np.concatenate([r["y"] for r in res.results])
    return out.reshape(_B, _C, _H, _W)


# revision 6
# speedup vs baseline: 1.1767x; 1.1767x over previous
"""Trainium2 Bass kernel for nn_DWTModelFullBand.

The reference computes a 2-level 2D Haar DWT (wavedec2) and immediately
inverts it (waverec2) reusing the cached level-1 detail bands. idwt2 is the
exact algebraic inverse of dwt2 (orthonormal Haar), so the whole pipeline is
the identity map on x; in fp32 the reference output differs from x only by
rounding noise (~6e-8 relative L2), the same magnitude any re-implementation
with different operation order would produce. The memory-roofline kernel is
therefore a pure copy: read x once from HBM, write it once.

Sharding: pure data parallel over batch — B=32 split as 4 samples per core
across 8 NeuronCores; each core DMA-copies its 12.58 MB shard DRAM->DRAM.

DMA schedule: the two HWDGE rings (Sync + Scalar queues) deal each
dma_start's descriptors to SDMA engines 0..rows-1 in order (the deal
restarts at engine 0 for every dma_start — verified from SDMA packet
traces). Engine 15 (E79) also hosts the ring processing and sustains only
~16 GB/s vs ~21 GB/s for the other engines, so a uniform spray makes it
straggle ~10 us. Each queue therefore mixes 16-row dma_starts (all engines)
with 15-row dma_starts (E15 skipped), sized so E15's total share is ~76% of
the other engines' and all 16 finish together. Element counts are chosen so
bass's AP splitter reproduces the intended row shapes: a 16*r chunk splits
into 16 rows of r for any r <= 16384; a 15*r chunk splits into 15 rows iff
16 does not divide r. Every dma_start carries .then_inc(sem, 16) (HWDGE
requires sync info; 16 four-byte packets, one per engine).
"""

import numpy as np

_B, _C, _H, _W = 32, 3, 512, 512
_NCORES = 8
_BS = _B // _NCORES  # batch shard per core
_SHARD_ELEMS = _BS * _C * _H * _W  # 3,145,728 f32 = 12.58 MB

# Per-queue stream: (rows, row_elems) per dma_start, laid out contiguously.
# E15 share = sum of 16-row r's = 76,029 elems; others also get the 15-row
# parts: + 356,400/15 = 23,760 elems -> ratio 0.762 ~= E15's relative rate.
_PARTS = [
    (16, 16384),
    (15, 11880),
    (16, 16384),
    (16, 16384),
    (15, 11880),
    (16, 16384),
    (16, 10493),
]
_QUEUE_ELEMS = sum(r * n for r, n in _PARTS)
assert 2 * _QUEUE_ELEMS == _SHARD_ELEMS
for _r, _n in _PARTS:
    assert _n <= 16384 and (_r == 16 or (_r == 15 and _n % 16 != 0))

_cache = {}


def _build_nc():
    import concourse.bass as bass
    import concourse.mybir as mybir

    nc = bass.Bass()
    x = nc.declare_dram_parameter("x", [_SHARD_ELEMS], mybir.dt.float32, isOutput=False)
    y = nc.declare_dram_parameter("y", [_SHARD_ELEMS], mybir.dt.float32, isOutput=True)

    n_dma = 0
    with nc.semaphore("dma_sem") as dma_sem:
        for qi, eng in enumerate((nc.sync, nc.scalar)):
            o = qi * _QUEUE_ELEMS
            for rows, row_elems in _PARTS:
                p = rows * row_elems
                sl = slice(o, o + p)
                eng.dma_start(out=y[sl], in_=x[sl]).then_inc(dma_sem, 16)
                o += p
                n_dma += 1
        nc.sync.wait_ge(dma_sem, 16 * n_dma)

    return nc


def _get_nc():
    if "nc" not in _cache:
        _cache["nc"] = _build_nc()
    return _cache["nc"]


def kernel(x: np.ndarray, *, _trace: bool = False, _tmpdir: str | None = None) -> np.ndarray:
    from concourse.bass_utils import run_bass_kernel_spmd

    x = np.ascontiguousarray(np.asarray(x), dtype=np.float32)
    assert x.shape == (_B, _C, _H, _W), x.shape

    nc = _get_nc()
    shards = x.reshape(_NCORES, _SHARD_ELEMS)
    in_maps = [{"x": shards[i]} for i in range(_NCORES)]
    res = run_bass_kernel_spmd(
        nc, in_maps, core_ids=list(range(_NCORES)), trace=_trace, tmpdir=_tmpdir
    )
    _cache["last_result"] = res
    out = np.concatenate([r["y"] for r in res.results])
    return out.reshape(_B, _C, _H, _W)
